# revision 1
# baseline (speedup 1.0000x reference)
"""Dense MLP forward (y = quantize(relu(x @ w + b))) on 8 TRN2 NeuronCores.

Strategy: pure data-parallel over the batch dim (1024 rows per core), w/b
replicated, no collectives. Host-side each core receives its x shard
*transposed* so the contraction dim lands on SBUF partitions with contiguous
DMA — zero on-chip transposes. Each core computes yT tiles:

  - matmuls in float32r (full PE rate at free-dim >= 256; measured faster
    than bf16, whose FWL weight loads steal rhs stream bandwidth),
    w chunks [128k,128n] stationary, xT chunks [128k,512m] moving,
    accumulating over k into all 8 PSUM banks, k-major wave order in band 0
    so the PE starts as soon as the first k-chunk lands; band 1 skewed so
    group stops stagger and evictions overlap matmuls.
  - w ships as int16 (values are 2^-16 fixed-point, |w*2^16| < 2^15) halving
    its HBM traffic; DVE expands it to f32r bit-exactly. y ships back bf16
    (matmul f32r noise ~1.3e-4 dwarfs nothing, bf16 out adds ~2.4e-3;
    harness tolerance is 2e-2) halving output traffic and the final store.
  - startup: the framework's entry all-engine barrier is deleted from the
    IR (every real dependency has a tile-emitted semaphore), so each engine
    flows straight from the NEFF wrapper into user code and the first input
    DMAs issue ~1.3us earlier. Junk matmuls (N=256, bf16) bridge the
    DMA-latency window and release the PE HAM clock throttle (1.2->2.4GHz)
    before real work begins; the first w chunk ships and expands in halves
    so the first 4 real matmuls start as early as possible.
  - epilogue per [128n, 512m] tile: relu(psum + b) in one op (bias is
    per-partition in the transposed layout), groups alternating ACT/DVE so
    PSUM banks release in parallel; each store gen is emitted right after
    its epilogue so no gen queues behind the final epilogues; the last two
    groups evict in halves (6 on DVE, 7 on ACT whose relu+bias op measures
    faster) with the four final half-stores alternating HWDGE rings.

Host transposes each core's yT back and concatenates.
"""

import numpy as np
import ml_dtypes

import concourse.bacc as bacc
import concourse.tile as tile
from concourse import mybir
from concourse.bass_utils import run_bass_kernel_spmd

P = 128
B, D_IN, D_OUT = 8192, 1024, 1024
N_CORES = 8
M = B // N_CORES          # batch rows per core
KC = D_IN // P            # 8 k-chunks
NT = D_OUT // P           # 8 n-groups (PSUM partition tiles)
MB = 512                  # matmul moving free dim / PSUM bank width (fp32)
NUM_MB = M // MB          # 2 m-bands per core

N_WARMUP_MM = 20          # N=256 junk MMs filling the first-DMA window
DEBARRIER = True          # drop the entry-block all-engine barrier

F32 = mybir.dt.float32
F32R = mybir.dt.float32r
BF16 = mybir.dt.bfloat16
I16 = mybir.dt.int16

_CACHE = {}


def build_bass(debarrier=DEBARRIER):
    nc = bacc.Bacc("TRN2", target_bir_lowering=False, debug=False)

    xT_d = nc.dram_tensor("xT", [D_IN, M], F32R, kind="ExternalInput")
    w_d = nc.dram_tensor("w", [D_IN, D_OUT], I16, kind="ExternalInput")
    b_d = nc.dram_tensor("b", [D_OUT], F32, kind="ExternalInput")
    yT_d = nc.dram_tensor("yT", [D_OUT, M], BF16, kind="ExternalOutput")

    with tile.TileContext(nc) as tc:
        with (
            tc.tile_pool(name="const", bufs=1) as cst,
            tc.tile_pool(name="wx", bufs=1) as wx,
            tc.tile_pool(name="outp", bufs=8) as outp,
            tc.tile_pool(name="ps", bufs=1, space="PSUM") as ps,
        ):
            w_tiles = [wx.tile([P, D_OUT], F32R, tag=f"wc{c}", name=f"wc{c}") for c in range(KC)]
            wi_tiles = [wx.tile([P, D_OUT], I16, tag=f"wic{c}", name=f"wic{c}") for c in range(KC)]
            x_tiles = [wx.tile([P, M], F32R, tag=f"xc{c}", name=f"xc{c}") for c in range(KC)]
            zt = cst.tile([P, 256], BF16, tag="warm_src")
            b_sb = cst.tile([P, NT], F32, tag="bias_raw")

            # ---- early ops ----
            # All inputs ride the SP ring (its completion sems fire promptly;
            # the ACT ring's were measured firing microseconds late at start).
            # First chunk in half-pieces so the first real MMs unblock ASAP.
            nc.gpsimd.memset(zt, 0.0)
            nc.sync.dma_start(out=wi_tiles[0][:, :MB], in_=w_d.ap()[:P, :MB])
            nc.sync.dma_start(out=x_tiles[0][:, :MB], in_=xT_d.ap()[:P, :MB])
            nc.sync.dma_start(out=wi_tiles[0][:, MB:], in_=w_d.ap()[:P, MB:])
            nc.sync.dma_start(out=wi_tiles[1], in_=w_d.ap()[P : 2 * P, :])
            nc.sync.dma_start(out=x_tiles[1][:, :MB], in_=xT_d.ap()[P : 2 * P, :MB])
            nc.scalar.dma_start(out=b_sb, in_=b_d.ap().rearrange("(c p) -> p c", p=P))

            # PE warm-up on junk data while the first input DMAs stream in
            warm_ps = ps.tile([P, MB], F32, tag="acc7")
            for _ in range(N_WARMUP_MM):
                nc.tensor.matmul(warm_ps[:, :256], zt[:, :P], zt, start=True, stop=True)

            # w0 expands in halves so the first real MMs unblock sooner;
            # w1 right after (its DMA is already in flight)
            nc.vector.tensor_scalar_mul(
                w_tiles[0][:, : MB], wi_tiles[0][:, : MB], 1.0 / 65536.0
            )
            nc.vector.tensor_scalar_mul(
                w_tiles[0][:, MB:], wi_tiles[0][:, MB:], 1.0 / 65536.0
            )
            nc.vector.tensor_scalar_mul(w_tiles[1], wi_tiles[1], 1.0 / 65536.0)

            # remaining inputs: w/x band-0 pieces interleaved on the SP
            # ring (the ACT ring's completion sems fire late and starve waves)
            for c in range(2, KC):
                nc.sync.dma_start(out=wi_tiles[c], in_=w_d.ap()[c * P : (c + 1) * P, :])
                nc.sync.dma_start(
                    out=x_tiles[c][:, :MB], in_=xT_d.ap()[c * P : (c + 1) * P, :MB]
                )
                nc.vector.tensor_scalar_mul(w_tiles[c], wi_tiles[c], 1.0 / 65536.0)
            for c in range(KC):
                nc.sync.dma_start(
                    out=x_tiles[c][:, MB:], in_=xT_d.ap()[c * P : (c + 1) * P, MB:]
                )

            def emit_mm(accs, mb, nt, c):
                nc.tensor.matmul(
                    accs[nt],
                    w_tiles[c][:, nt * P : (nt + 1) * P],
                    x_tiles[c][:, mb * MB : (mb + 1) * MB],
                    start=(c == 0),
                    stop=(c == KC - 1),
                )

            def emit_epi(acc_sl, nt, on_act, o_sl):
                # relu(psum + b) -> bf16; bias varies along partitions here
                if on_act:
                    nc.scalar.activation(
                        o_sl, acc_sl, mybir.ActivationFunctionType.Relu,
                        bias=b_sb[:, nt : nt + 1], scale=1.0,
                    )
                else:
                    nc.vector.tensor_scalar(
                        o_sl, acc_sl, b_sb[:, nt : nt + 1], 0.0,
                        mybir.AluOpType.add, mybir.AluOpType.max,
                    )

            # ---- band 0: k-major waves, 8 MMs per arriving chunk ----
            accs = [ps.tile([P, MB], F32, tag=f"acc{nt}", name=f"acc{nt}") for nt in range(NT)]
            for c in range(KC):
                for nt in range(NT):
                    emit_mm(accs, 0, nt, c)
            otiles = []
            for nt in range(NT):
                o = outp.tile([P, MB], BF16, tag="otile")
                otiles.append(o)
                emit_epi(accs[nt], nt, nt % 2 == 0, o)
            for nt in range(NT):
                ring = nc.scalar if nt % 2 == 0 else nc.sync
                ring.dma_start(
                    out=yT_d.ap()[nt * P : (nt + 1) * P, :MB], in_=otiles[nt]
                )

            # ---- band 1: skewed waves (group nt runs chunk c at wave
            # t=nt+c) so stops stagger ~8 MMs apart and evictions overlap
            # matmuls; the last two groups evict in halves (6 on DVE, 7 on
            # ACT whose relu+bias op is measured faster), stores alternating
            # rings so the post-last-matmul chain is short. ----
            accs = [ps.tile([P, MB], F32, tag=f"acc{nt}", name=f"acc{nt}") for nt in range(NT)]
            H = MB // 2
            for t in range(KC + NT - 1):
                for nt in range(NT):
                    c = t - nt
                    if 0 <= c < KC:
                        emit_mm(accs, 1, nt, c)
            # epi+store emitted per group so no store gen queues behind the
            # final epilogues; groups 6 (DVE) and 7 (ACT, faster op) evict in
            # halves with the four final half-stores alternating rings
            otiles = [outp.tile([P, MB], BF16, tag="otile", name=f"ot1_{i}") for i in range(NT)]
            for nt in range(NT - 2):
                emit_epi(accs[nt], nt, nt % 2 == 0, otiles[nt])
                ring = nc.sync if nt % 2 == 0 else nc.scalar
                ring.dma_start(
                    out=yT_d.ap()[nt * P : (nt + 1) * P, MB:], in_=otiles[nt]
                )
            emit_epi(accs[6], 6, False, otiles[6])
            nc.sync.dma_start(out=yT_d.ap()[6 * P : 7 * P, MB:], in_=otiles[6])
            emit_epi(accs[7], 7, True, otiles[7])
            nc.sync.dma_start(out=yT_d.ap()[7 * P :, MB:], in_=otiles[7])

    if debarrier:
        # Drop the framework's entry all-engine barrier: every real
        # dependency already has a tile-emitted semaphore, and the barrier
        # serializes all engines behind the slowest pre-barrier stream.
        entry = nc.main_func.blocks[0]
        drop = [
            inst for inst in entry.instructions
            if type(inst).__name__ in ("InstDrain", "InstEventSemaphore")
        ]
        assert len(drop) == 11, [str(i)[:60] for i in drop]
        n_bar = sum("barrier_" in str(i) for i in drop)
        assert n_bar == 10, n_bar  # 5x(Drain+EvtSem w/ barrier sem) + bare PL Drain
        for inst in drop:
            entry.instructions.remove(inst)

    nc.compile()
    return nc


def get_nc():
    if "nc" not in _CACHE:
        _CACHE["nc"] = build_bass()
    return _CACHE["nc"]


def make_in_maps(x, w, b):
    x = np.ascontiguousarray(x, dtype=np.float32)
    w = np.asarray(w, dtype=np.float32)
    b = np.ascontiguousarray(b, dtype=np.float32)
    # w lives on the 2^-16 fixed-point grid with |w| < 0.5, so w*2^16 is an
    # int16-exact integer; ship it at half the bytes and expand on-chip.
    w_int = np.round(w * 65536.0)
    assert np.abs(w_int).max() < 32768 and np.array_equal(
        w_int.astype(np.float32) / 65536.0, w
    ), "w does not fit the int16 fixed-point fast path"
    w_i16 = np.ascontiguousarray(w_int.astype(np.int16))
    xs = x.reshape(N_CORES, M, D_IN)
    return [
        {"xT": np.ascontiguousarray(xs[i].T), "w": w_i16, "b": b}
        for i in range(N_CORES)
    ]


def gather_out(results):
    return np.concatenate(
        [results[i]["yT"].astype(np.float32).T for i in range(N_CORES)], axis=0
    )


def kernel(x, w, b):
    nc = get_nc()
    res = run_bass_kernel_spmd(nc, make_in_maps(x, w, b), core_ids=list(range(N_CORES)))
    return gather_out(res.results)



# revision 2
# speedup vs baseline: 1.0488x; 1.0488x over previous
"""Dense MLP forward (y = quantize(relu(x @ w + b))) on 8 TRN2 NeuronCores.

Strategy: pure data-parallel over the batch dim (1024 rows per core), w/b
replicated, no collectives. Host-side each core receives its x shard
*transposed* so the contraction dim lands on SBUF partitions with contiguous
DMA — zero on-chip transposes. Each core computes yT tiles:

  - matmuls in bf16 (x and w both rounded host-side; adds ~3e-3 rel err vs
    the 2e-2 gate). bf16 halves x's HBM traffic vs f32 and LDWEIGHTS gets
    FWL (4-xbus fast weight load), so the LDW fully hides under the 213ns
    N=512 moving stream; w chunks [128k,128n] stationary, xT chunks
    [128k,512m] moving, accumulating over k into all 8 PSUM banks; k-major
    wave order in band 0 so the PE starts as soon as the first chunks land;
    band 1 skewed so group stops stagger and evictions overlap matmuls.
  - startup: the framework's entry all-engine barrier is deleted from the
    IR; junk matmuls (N=256, bf16, first thing the PE runs) bridge the
    first-DMA window and release the PE HAM clock throttle (1.2->2.4GHz).
    Input DMAs split across the SP ring (x band-0 + first w pieces,
    interleaved) and ACT ring (bias + later w chunks) so descriptor
    generation (~0.6us per dma_start, serialized per sequencer) doesn't
    gate chunk arrival.
  - epilogue per [128n, 512m] tile: relu(psum + b) in one op (bias is
    per-partition in the transposed layout), groups alternating ACT/DVE so
    PSUM banks release in parallel; each store gen is emitted right after
    its epilogue. The last two groups run their epilogues in half-width
    pieces on BOTH engines concurrently and store the halves on separate
    DMA rings, shortening the post-last-matmul chain.

Host transposes each core's yT back and concatenates.
"""

import numpy as np
import ml_dtypes

import concourse.bacc as bacc
import concourse.tile as tile
from concourse import mybir
from concourse.bass_utils import run_bass_kernel_spmd

P = 128
B, D_IN, D_OUT = 8192, 1024, 1024
N_CORES = 8
M = B // N_CORES          # batch rows per core
KC = D_IN // P            # 8 k-chunks
NT = D_OUT // P           # 8 n-groups (PSUM partition tiles)
MB = 512                  # matmul moving free dim / PSUM bank width (fp32)
NUM_MB = M // MB          # 2 m-bands per core

N_WARMUP_MM = 12          # N=256 junk MMs filling the first-DMA window
DEBARRIER = True          # drop the entry-block all-engine barrier

F32 = mybir.dt.float32
BF16 = mybir.dt.bfloat16

_CACHE = {}


def build_bass(debarrier=DEBARRIER):
    nc = bacc.Bacc("TRN2", target_bir_lowering=False, debug=False)

    xT_d = nc.dram_tensor("xT", [D_IN, M], BF16, kind="ExternalInput")
    w_d = nc.dram_tensor("w", [D_IN, D_OUT], BF16, kind="ExternalInput")
    b_d = nc.dram_tensor("b", [D_OUT], F32, kind="ExternalInput")
    yT_d = nc.dram_tensor("yT", [D_OUT, M], BF16, kind="ExternalOutput")

    with tile.TileContext(nc) as tc:
        with (
            tc.tile_pool(name="const", bufs=1) as cst,
            tc.tile_pool(name="wx", bufs=1) as wx,
            tc.tile_pool(name="outp", bufs=8) as outp,
            tc.tile_pool(name="ps", bufs=1, space="PSUM") as ps,
        ):
            w_tiles = [wx.tile([P, D_OUT], BF16, tag=f"wc{c}", name=f"wc{c}") for c in range(KC)]
            x_tiles = [wx.tile([P, M], BF16, tag=f"xc{c}", name=f"xc{c}") for c in range(KC)]
            zt = cst.tile([P, 256], BF16, tag="warm_src")
            b_sb = cst.tile([P, NT], F32, tag="bias_raw")

            # ---- early ops ----
            # zt memset is the first Pool op so the PE warm-up can begin the
            # moment the engines come out of the runtime preamble.
            nc.gpsimd.memset(zt, 0.0)
            # SP ring: first w pieces + x band-0 chunks, interleaved so the
            # wave-c inputs land just ahead of the PE's k-major schedule.
            nc.sync.dma_start(out=w_tiles[0][:, :MB], in_=w_d.ap()[:P, :MB])
            nc.sync.dma_start(out=x_tiles[0][:, :MB], in_=xT_d.ap()[:P, :MB])
            nc.sync.dma_start(out=w_tiles[0][:, MB:], in_=w_d.ap()[:P, MB:])
            nc.sync.dma_start(out=x_tiles[1][:, :MB], in_=xT_d.ap()[P : 2 * P, :MB])
            nc.sync.dma_start(out=w_tiles[1], in_=w_d.ap()[P : 2 * P, :])
            # ACT ring: bias + the later w chunks (needed from wave 2 on).
            nc.scalar.dma_start(out=b_sb, in_=b_d.ap().rearrange("(c p) -> p c", p=P))
            for c in range(2, KC):
                nc.scalar.dma_start(out=w_tiles[c], in_=w_d.ap()[c * P : (c + 1) * P, :])

            # PE warm-up on junk data while the first input DMAs stream in
            warm_ps = ps.tile([P, MB], F32, tag="acc7")
            for _ in range(N_WARMUP_MM):
                nc.tensor.matmul(warm_ps[:, :256], zt[:, :P], zt, start=True, stop=True)

            # remaining x band-0 pieces, then band-1 pieces, all on SP
            for c in range(2, KC):
                nc.sync.dma_start(
                    out=x_tiles[c][:, :MB], in_=xT_d.ap()[c * P : (c + 1) * P, :MB]
                )
            for c in range(KC):
                nc.sync.dma_start(
                    out=x_tiles[c][:, MB:], in_=xT_d.ap()[c * P : (c + 1) * P, MB:]
                )

            def emit_mm(accs, mb, nt, c):
                nc.tensor.matmul(
                    accs[nt],
                    w_tiles[c][:, nt * P : (nt + 1) * P],
                    x_tiles[c][:, mb * MB : (mb + 1) * MB],
                    start=(c == 0),
                    stop=(c == KC - 1),
                )

            def emit_epi(acc_sl, nt, on_act, o_sl):
                # relu(psum + b) -> bf16; bias varies along partitions here
                if on_act:
                    nc.scalar.activation(
                        o_sl, acc_sl, mybir.ActivationFunctionType.Relu,
                        bias=b_sb[:, nt : nt + 1], scale=1.0,
                    )
                else:
                    nc.vector.tensor_scalar(
                        o_sl, acc_sl, b_sb[:, nt : nt + 1], 0.0,
                        mybir.AluOpType.add, mybir.AluOpType.max,
                    )

            # ---- band 0: k-major waves, 8 MMs per arriving chunk ----
            accs = [ps.tile([P, MB], F32, tag=f"acc{nt}", name=f"acc{nt}") for nt in range(NT)]
            for c in range(KC):
                for nt in range(NT):
                    emit_mm(accs, 0, nt, c)
            otiles = []
            for nt in range(NT):
                o = outp.tile([P, MB], BF16, tag="otile")
                otiles.append(o)
                emit_epi(accs[nt], nt, nt % 2 == 0, o)
            for nt in range(NT):
                ring = nc.scalar if nt % 2 == 0 else nc.sync
                ring.dma_start(
                    out=yT_d.ap()[nt * P : (nt + 1) * P, :MB], in_=otiles[nt]
                )

            # ---- band 1: skewed waves (group nt runs chunk c at wave
            # t=nt+c) so stops stagger and evictions overlap matmuls; the
            # last two groups split their epilogues across ACT+DVE and their
            # stores across rings so the post-last-matmul chain is short. ----
            accs = [ps.tile([P, MB], F32, tag=f"acc{nt}", name=f"acc{nt}") for nt in range(NT)]
            H = MB // 2
            for t in range(KC + NT - 1):
                for nt in range(NT):
                    c = t - nt
                    if 0 <= c < KC:
                        emit_mm(accs, 1, nt, c)
            otiles = [outp.tile([P, MB], BF16, tag="otile", name=f"ot1_{i}") for i in range(NT)]
            for nt in range(NT - 2):
                emit_epi(accs[nt], nt, nt % 2 == 0, otiles[nt])
                ring = nc.sync if nt % 2 == 0 else nc.scalar
                ring.dma_start(
                    out=yT_d.ap()[nt * P : (nt + 1) * P, MB:], in_=otiles[nt]
                )
            # group 6: halves on DVE+ACT, stores on sync+scalar rings
            emit_epi(accs[6][:, :H], 6, False, otiles[6][:, :H])
            emit_epi(accs[6][:, H:], 6, True, otiles[6][:, H:])
            nc.sync.dma_start(out=yT_d.ap()[6 * P : 7 * P, MB : MB + H], in_=otiles[6][:, :H])
            nc.scalar.dma_start(out=yT_d.ap()[6 * P : 7 * P, MB + H :], in_=otiles[6][:, H:])
            # group 7 (the true last): halves on ACT+DVE, stores on the two
            # rings whose queues are free at that point
            emit_epi(accs[7][:, :H], 7, True, otiles[7][:, :H])
            emit_epi(accs[7][:, H:], 7, False, otiles[7][:, H:])
            nc.scalar.dma_start(out=yT_d.ap()[7 * P :, MB : MB + H], in_=otiles[7][:, :H])
            nc.sync.dma_start(out=yT_d.ap()[7 * P :, MB + H :], in_=otiles[7][:, H:])

    if debarrier:
        # Drop the framework's entry all-engine barrier: every real
        # dependency already has a tile-emitted semaphore, and the barrier
        # serializes all engines behind the slowest pre-barrier stream.
        entry = nc.main_func.blocks[0]
        drop = [
            inst for inst in entry.instructions
            if type(inst).__name__ in ("InstDrain", "InstEventSemaphore")
        ]
        assert len(drop) == 11, [str(i)[:60] for i in drop]
        n_bar = sum("barrier_" in str(i) for i in drop)
        assert n_bar == 10, n_bar  # 5x(Drain+EvtSem w/ barrier sem) + bare PL Drain
        for inst in drop:
            entry.instructions.remove(inst)

    nc.compile()
    return nc


def get_nc():
    if "nc" not in _CACHE:
        _CACHE["nc"] = build_bass()
    return _CACHE["nc"]


def make_in_maps(x, w, b):
    x = np.asarray(x, dtype=np.float32)
    w = np.asarray(w, dtype=np.float32)
    b = np.ascontiguousarray(b, dtype=np.float32)
    w_bf = np.ascontiguousarray(w.astype(ml_dtypes.bfloat16))
    xs = x.reshape(N_CORES, M, D_IN)
    return [
        {
            "xT": np.ascontiguousarray(xs[i].T.astype(ml_dtypes.bfloat16)),
            "w": w_bf,
            "b": b,
        }
        for i in range(N_CORES)
    ]


def gather_out(results):
    return np.concatenate(
        [results[i]["yT"].astype(np.float32).T for i in range(N_CORES)], axis=0
    )


def kernel(x, w, b):
    nc = get_nc()
    res = run_bass_kernel_spmd(nc, make_in_maps(x, w, b), core_ids=list(range(N_CORES)))
    return gather_out(res.results)


# revision 5
# speedup vs baseline: 1.0803x; 1.0301x over previous
"""Dense MLP forward (y = quantize(relu(x @ w + b))) on 8 TRN2 NeuronCores.

Strategy: pure data-parallel over the batch dim (1024 rows per core), w/b
replicated, no collectives. Host-side each core receives its x shard
*transposed* so the contraction dim lands on SBUF partitions with contiguous
DMA — zero on-chip transposes. Each core computes yT tiles:

  - matmuls in bf16 (x and w both rounded host-side; adds ~3e-3 rel err vs
    the 2e-2 gate). bf16 halves x's HBM traffic vs f32 and LDWEIGHTS gets
    FWL (4-xbus fast weight load), so the LDW fully hides under the 213ns
    N=512 moving stream; w chunks [128k,128n] stationary, xT chunks
    [128k,512m] moving, accumulating over k into all 8 PSUM banks; k-major
    wave order in band 0 so the PE starts as soon as the first chunks land;
    band 1 skewed so group stops stagger and evictions overlap matmuls.
  - startup: the framework's entry all-engine barrier is deleted from the
    IR; junk matmuls (N=256, bf16, first thing the PE runs) bridge the
    first-DMA window and release the PE HAM clock throttle (1.2->2.4GHz).
    Input DMAs split across the SP ring (x band-0 + first w pieces,
    interleaved) and ACT ring (bias + later w chunks) so descriptor
    generation (~0.6us per dma_start, serialized per sequencer) doesn't
    gate chunk arrival.
  - epilogue per [128n, 512m] tile: relu(psum + b) in one op (bias is
    per-partition in the transposed layout), groups alternating ACT/DVE so
    PSUM banks release in parallel; each store gen is emitted right after
    its epilogue. The last two groups run their epilogues in half-width
    pieces on BOTH engines concurrently and store the halves on separate
    DMA rings, shortening the post-last-matmul chain.

Host transposes each core's yT back and concatenates.
"""

import numpy as np
import ml_dtypes

import concourse.bacc as bacc
import concourse.tile as tile
from concourse import mybir
from concourse.bass_utils import run_bass_kernel_spmd

P = 128
B, D_IN, D_OUT = 8192, 1024, 1024
N_CORES = 8
M = B // N_CORES          # batch rows per core
KC = D_IN // P            # 8 k-chunks
NT = D_OUT // P           # 8 n-groups (PSUM partition tiles)
MB = 512                  # matmul moving free dim / PSUM bank width (fp32)
NUM_MB = M // MB          # 2 m-bands per core

N_WARMUP_MM = 14          # N=256 junk MMs filling the first-DMA window
DEBARRIER = True          # drop the entry-block all-engine barrier

F32 = mybir.dt.float32
BF16 = mybir.dt.bfloat16

_CACHE = {}


def build_bass(debarrier=DEBARRIER):
    nc = bacc.Bacc("TRN2", target_bir_lowering=False, debug=False)

    xT_d = nc.dram_tensor("xT", [D_IN, M], BF16, kind="ExternalInput")
    w_d = nc.dram_tensor("w", [D_IN, D_OUT], BF16, kind="ExternalInput")
    b_d = nc.dram_tensor("b", [D_OUT], F32, kind="ExternalInput")
    yT_d = nc.dram_tensor("yT", [D_OUT, M], BF16, kind="ExternalOutput")

    with tile.TileContext(nc) as tc:
        with (
            tc.tile_pool(name="const", bufs=1) as cst,
            tc.tile_pool(name="wx", bufs=1) as wx,
            tc.tile_pool(name="outp", bufs=16) as outp,
            tc.tile_pool(name="ps", bufs=1, space="PSUM") as ps,
        ):
            w_tiles = [wx.tile([P, D_OUT], BF16, tag=f"wc{c}", name=f"wc{c}") for c in range(KC)]
            x_tiles = [wx.tile([P, M], BF16, tag=f"xc{c}", name=f"xc{c}") for c in range(KC)]
            zt = cst.tile([P, 256], BF16, tag="warm_src")
            b_sb = cst.tile([P, NT], F32, tag="bias_raw")

            # ---- early ops ----
            # zt memset is the first Pool op so the PE warm-up can begin the
            # moment the engines come out of the runtime preamble.
            nc.gpsimd.memset(zt, 0.0)
            # The ACT ring's HWDGE queue starts flowing ~2us earlier than the
            # SP ring's (measured), so the stream-gating pieces (w0 halves,
            # x0 band 0, w1) go on ACT, followed by the later w chunks and
            # the bias. SP carries all remaining x pieces.
            nc.scalar.dma_start(out=w_tiles[0][:, :MB], in_=w_d.ap()[:P, :MB])
            nc.scalar.dma_start(out=x_tiles[0][:, :MB], in_=xT_d.ap()[:P, :MB])
            nc.scalar.dma_start(out=w_tiles[0][:, MB:], in_=w_d.ap()[:P, MB:])
            nc.scalar.dma_start(out=w_tiles[1], in_=w_d.ap()[P : 2 * P, :])
            for c in range(2, KC):
                nc.scalar.dma_start(out=w_tiles[c], in_=w_d.ap()[c * P : (c + 1) * P, :])
            nc.scalar.dma_start(out=b_sb, in_=b_d.ap().rearrange("(c p) -> p c", p=P))

            # PE warm-up on junk data while the first input DMAs stream in
            warm_ps = ps.tile([P, MB], F32, tag="acc7")
            for _ in range(N_WARMUP_MM):
                nc.tensor.matmul(warm_ps[:, :256], zt[:, :P], zt, start=True, stop=True)

            # x band-0 pieces c>=1, then band-1 pieces, all on SP
            for c in range(1, KC):
                nc.sync.dma_start(
                    out=x_tiles[c][:, :MB], in_=xT_d.ap()[c * P : (c + 1) * P, :MB]
                )
            for c in range(KC):
                nc.sync.dma_start(
                    out=x_tiles[c][:, MB:], in_=xT_d.ap()[c * P : (c + 1) * P, MB:]
                )

            def emit_mm(accs, mb, nt, c):
                nc.tensor.matmul(
                    accs[nt],
                    w_tiles[c][:, nt * P : (nt + 1) * P],
                    x_tiles[c][:, mb * MB : (mb + 1) * MB],
                    start=(c == 0),
                    stop=(c == KC - 1),
                )

            def emit_epi(acc_sl, nt, on_act, o_sl):
                # relu(psum + b) -> bf16; bias varies along partitions here
                if on_act:
                    nc.scalar.activation(
                        o_sl, acc_sl, mybir.ActivationFunctionType.Relu,
                        bias=b_sb[:, nt : nt + 1], scale=1.0,
                    )
                else:
                    nc.vector.tensor_scalar(
                        o_sl, acc_sl, b_sb[:, nt : nt + 1], 0.0,
                        mybir.AluOpType.add, mybir.AluOpType.max,
                    )

            # ---- band 0: k-major waves, 8 MMs per arriving chunk ----
            accs = [ps.tile([P, MB], F32, tag=f"acc{nt}", name=f"acc{nt}") for nt in range(NT)]
            for c in range(KC):
                for nt in range(NT):
                    emit_mm(accs, 0, nt, c)
            otiles = []
            for nt in range(NT):
                o = outp.tile([P, MB], BF16, tag="otile")
                otiles.append(o)
                emit_epi(accs[nt], nt, nt % 2 == 0, o)
            for nt in range(NT):
                ring = nc.scalar if nt % 2 == 0 else nc.sync
                ring.dma_start(
                    out=yT_d.ap()[nt * P : (nt + 1) * P, :MB], in_=otiles[nt]
                )

            # ---- band 1: skewed waves (group nt runs chunk c at wave
            # t=nt+c) so stops stagger and evictions overlap matmuls; the
            # last two groups split their epilogues across ACT+DVE and their
            # stores across rings so the post-last-matmul chain is short. ----
            accs = [ps.tile([P, MB], F32, tag=f"acc{nt}", name=f"acc{nt}") for nt in range(NT)]
            H = MB // 2
            for t in range(KC + NT - 1):
                for nt in range(NT):
                    c = t - nt
                    if 0 <= c < KC:
                        emit_mm(accs, 1, nt, c)
            otiles = [outp.tile([P, MB], BF16, tag="otile", name=f"ot1_{i}") for i in range(NT)]
            for nt in range(NT - 2):
                emit_epi(accs[nt], nt, nt % 2 == 0, otiles[nt])
                ring = nc.sync if nt % 2 == 0 else nc.scalar
                ring.dma_start(
                    out=yT_d.ap()[nt * P : (nt + 1) * P, MB:], in_=otiles[nt]
                )
            # group 6: halves on DVE+ACT, stores on sync+scalar rings
            emit_epi(accs[6][:, :H], 6, False, otiles[6][:, :H])
            emit_epi(accs[6][:, H:], 6, True, otiles[6][:, H:])
            nc.sync.dma_start(out=yT_d.ap()[6 * P : 7 * P, MB : MB + H], in_=otiles[6][:, :H])
            nc.scalar.dma_start(out=yT_d.ap()[6 * P : 7 * P, MB + H :], in_=otiles[6][:, H:])
            # group 7 (the true last): halves on ACT+DVE, stores on the two
            # rings whose queues are free at that point
            emit_epi(accs[7][:, :H], 7, True, otiles[7][:, :H])
            emit_epi(accs[7][:, H:], 7, False, otiles[7][:, H:])
            nc.scalar.dma_start(out=yT_d.ap()[7 * P :, MB : MB + H], in_=otiles[7][:, :H])
            nc.sync.dma_start(out=yT_d.ap()[7 * P :, MB + H :], in_=otiles[7][:, H:])

    if debarrier:
        # Drop the framework's entry all-engine barrier: every real
        # dependency already has a tile-emitted semaphore, and the barrier
        # serializes all engines behind the slowest pre-barrier stream.
        entry = nc.main_func.blocks[0]
        drop = [
            inst for inst in entry.instructions
            if type(inst).__name__ in ("InstDrain", "InstEventSemaphore")
        ]
        assert len(drop) == 11, [str(i)[:60] for i in drop]
        n_bar = sum("barrier_" in str(i) for i in drop)
        assert n_bar == 10, n_bar  # 5x(Drain+EvtSem w/ barrier sem) + bare PL Drain
        for inst in drop:
            entry.instructions.remove(inst)

    nc.compile()
    return nc


def get_nc():
    if "nc" not in _CACHE:
        _CACHE["nc"] = build_bass()
    return _CACHE["nc"]


def make_in_maps(x, w, b):
    x = np.asarray(x, dtype=np.float32)
    w = np.asarray(w, dtype=np.float32)
    b = np.ascontiguousarray(b, dtype=np.float32)
    w_bf = np.ascontiguousarray(w.astype(ml_dtypes.bfloat16))
    xs = x.reshape(N_CORES, M, D_IN)
    return [
        {
            "xT": np.ascontiguousarray(xs[i].T.astype(ml_dtypes.bfloat16)),
            "w": w_bf,
            "b": b,
        }
        for i in range(N_CORES)
    ]


def gather_out(results):
    return np.concatenate(
        [results[i]["yT"].astype(np.float32).T for i in range(N_CORES)], axis=0
    )


def kernel(x, w, b):
    nc = get_nc()
    res = run_bass_kernel_spmd(nc, make_in_maps(x, w, b), core_ids=list(range(N_CORES)))
    return gather_out(res.results)


# revision 9
# speedup vs baseline: 1.0942x; 1.0129x over previous
"""Dense MLP forward (y = quantize(relu(x @ w + b))) on 8 TRN2 NeuronCores.

Strategy: pure data-parallel over the batch dim (1024 rows per core), w/b
replicated, no collectives. Host-side each core receives its x shard
*transposed* so the contraction dim lands on SBUF partitions with contiguous
DMA — zero on-chip transposes. Each core computes yT tiles:

  - matmuls in bf16 (x and w both rounded host-side; adds ~3e-3 rel err vs
    the 2e-2 gate). bf16 halves x's HBM traffic vs f32 and LDWEIGHTS gets
    FWL (4-xbus fast weight load), so the LDW fully hides under the 213ns
    N=512 moving stream; w chunks [128k,128n] stationary, xT chunks
    [128k,512m] moving, accumulating over k into all 8 PSUM banks; k-major
    wave order in band 0 so the PE starts as soon as the first chunks land;
    band 1 skewed so group stops stagger and evictions overlap matmuls.
  - startup: the framework's entry all-engine barrier is deleted from the
    IR; junk matmuls (N=256, bf16, first thing the PE runs) bridge the
    first-DMA window and release the PE HAM clock throttle (1.2->2.4GHz).
    Input DMAs split across the SP ring (x band-0 + first w pieces,
    interleaved) and ACT ring (bias + later w chunks) so descriptor
    generation (~0.6us per dma_start, serialized per sequencer) doesn't
    gate chunk arrival.
  - epilogue per [128n, 512m] tile: relu(psum + b) in one op (bias is
    per-partition in the transposed layout), groups alternating ACT/DVE so
    PSUM banks release in parallel; each store gen is emitted right after
    its epilogue. The last two groups run their epilogues in half-width
    pieces on BOTH engines concurrently and store the halves on separate
    DMA rings, shortening the post-last-matmul chain.

Host transposes each core's yT back and concatenates.
"""

import numpy as np
import ml_dtypes

import concourse.bacc as bacc
import concourse.tile as tile
from concourse import mybir
from concourse.bass_utils import run_bass_kernel_spmd

P = 128
B, D_IN, D_OUT = 8192, 1024, 1024
N_CORES = 8
M = B // N_CORES          # batch rows per core
KC = D_IN // P            # 8 k-chunks
NT = D_OUT // P           # 8 n-groups (PSUM partition tiles)
MB = 512                  # matmul moving free dim / PSUM bank width (fp32)
NUM_MB = M // MB          # 2 m-bands per core

N_WARMUP_MM = 12          # N=256 junk MMs filling the first-DMA window
DEBARRIER = True          # drop the entry-block all-engine barrier

F32 = mybir.dt.float32
BF16 = mybir.dt.bfloat16

_CACHE = {}


def build_bass(debarrier=DEBARRIER):
    nc = bacc.Bacc("TRN2", target_bir_lowering=False, debug=False)

    xT_d = nc.dram_tensor("xT", [D_IN, M], BF16, kind="ExternalInput")
    w_d = nc.dram_tensor("w", [D_IN, D_OUT], BF16, kind="ExternalInput")
    b_d = nc.dram_tensor("b", [D_OUT], F32, kind="ExternalInput")
    yT_d = nc.dram_tensor("yT", [D_OUT, M], BF16, kind="ExternalOutput")

    with tile.TileContext(nc) as tc:
        with (
            tc.tile_pool(name="const", bufs=1) as cst,
            tc.tile_pool(name="wx", bufs=1) as wx,
            tc.tile_pool(name="outp", bufs=16) as outp,
            tc.tile_pool(name="ps", bufs=1, space="PSUM") as ps,
        ):
            w_tiles = [wx.tile([P, D_OUT], BF16, tag=f"wc{c}", name=f"wc{c}") for c in range(KC)]
            x_tiles = [wx.tile([P, M], BF16, tag=f"xc{c}", name=f"xc{c}") for c in range(KC)]
            zt = cst.tile([P, 256], BF16, tag="warm_src")
            b_sb = cst.tile([P, NT], F32, tag="bias_raw")

            # ---- early ops ----
            # zt memset is the first Pool op so the PE warm-up can begin the
            # moment the engines come out of the runtime preamble.
            nc.gpsimd.memset(zt, 0.0)
            # A single HWDGE queue moves only ~135 GB/s while several are
            # active, so inputs spread across THREE queues in need-order:
            # ACT (starts flowing earliest) takes w0 halves + even w chunks,
            # the gpsimd SWDGE queue takes x0 band-0 + odd w chunks (gpsimd
            # leaves the runtime preamble first), SP (starts ~1.3us later)
            # takes the remaining x pieces whose waves come later.
            nc.scalar.dma_start(out=w_tiles[0][:, :MB], in_=w_d.ap()[:P, :MB])
            nc.scalar.dma_start(out=w_tiles[0][:, MB:], in_=w_d.ap()[:P, MB:])
            nc.gpsimd.dma_start(out=x_tiles[0][:, :MB], in_=xT_d.ap()[:P, :MB])
            nc.gpsimd.dma_start(out=w_tiles[1], in_=w_d.ap()[P : 2 * P, :])
            for c in range(2, KC):
                ring = nc.scalar if c % 2 == 0 else nc.gpsimd
                ring.dma_start(out=w_tiles[c], in_=w_d.ap()[c * P : (c + 1) * P, :])
            nc.scalar.dma_start(out=b_sb, in_=b_d.ap().rearrange("(c p) -> p c", p=P))

            # PE warm-up on junk data while the first input DMAs stream in
            warm_ps = ps.tile([P, MB], F32, tag="acc7")
            for _ in range(N_WARMUP_MM):
                nc.tensor.matmul(warm_ps[:, :256], zt[:, :P], zt, start=True, stop=True)

            # x band-0 pieces c>=1, then band-1 pieces, all on SP
            for c in range(1, KC):
                nc.sync.dma_start(
                    out=x_tiles[c][:, :MB], in_=xT_d.ap()[c * P : (c + 1) * P, :MB]
                )
            for c in range(KC):
                nc.sync.dma_start(
                    out=x_tiles[c][:, MB:], in_=xT_d.ap()[c * P : (c + 1) * P, MB:]
                )

            def emit_mm(accs, mb, nt, c):
                nc.tensor.matmul(
                    accs[nt],
                    w_tiles[c][:, nt * P : (nt + 1) * P],
                    x_tiles[c][:, mb * MB : (mb + 1) * MB],
                    start=(c == 0),
                    stop=(c == KC - 1),
                )

            def emit_epi(acc_sl, nt, on_act, o_sl):
                # relu(psum + b) -> bf16; bias varies along partitions here
                if on_act:
                    nc.scalar.activation(
                        o_sl, acc_sl, mybir.ActivationFunctionType.Relu,
                        bias=b_sb[:, nt : nt + 1], scale=1.0,
                    )
                else:
                    nc.vector.tensor_scalar(
                        o_sl, acc_sl, b_sb[:, nt : nt + 1], 0.0,
                        mybir.AluOpType.add, mybir.AluOpType.max,
                    )

            # ---- band 0: k-major waves, 8 MMs per arriving chunk ----
            accs = [ps.tile([P, MB], F32, tag=f"acc{nt}", name=f"acc{nt}") for nt in range(NT)]
            for c in range(KC):
                for nt in range(NT):
                    emit_mm(accs, 0, nt, c)
            otiles = []
            for nt in range(NT):
                o = outp.tile([P, MB], BF16, tag="otile")
                otiles.append(o)
                emit_epi(accs[nt], nt, nt % 2 == 0, o)
            for nt in range(NT):
                ring = nc.scalar if nt % 2 == 0 else nc.sync
                ring.dma_start(
                    out=yT_d.ap()[nt * P : (nt + 1) * P, :MB], in_=otiles[nt]
                )

            # ---- band 1: skewed waves (group nt runs chunk c at wave
            # t=nt+c) so stops stagger and evictions overlap matmuls; the
            # last two groups split their epilogues across ACT+DVE and their
            # stores across rings so the post-last-matmul chain is short. ----
            accs = [ps.tile([P, MB], F32, tag=f"acc{nt}", name=f"acc{nt}") for nt in range(NT)]
            H = MB // 2
            for t in range(KC + NT - 1):
                for nt in range(NT):
                    c = t - nt
                    if 0 <= c < KC:
                        emit_mm(accs, 1, nt, c)
            otiles = [outp.tile([P, MB], BF16, tag="otile", name=f"ot1_{i}") for i in range(NT)]
            for nt in range(NT - 2):
                emit_epi(accs[nt], nt, nt % 2 == 0, otiles[nt])
                ring = nc.sync if nt % 2 == 0 else nc.scalar
                ring.dma_start(
                    out=yT_d.ap()[nt * P : (nt + 1) * P, MB:], in_=otiles[nt]
                )
            # group 6: halves on DVE+ACT, stores on sync+scalar rings
            emit_epi(accs[6][:, :H], 6, False, otiles[6][:, :H])
            emit_epi(accs[6][:, H:], 6, True, otiles[6][:, H:])
            nc.sync.dma_start(out=yT_d.ap()[6 * P : 7 * P, MB : MB + H], in_=otiles[6][:, :H])
            nc.scalar.dma_start(out=yT_d.ap()[6 * P : 7 * P, MB + H :], in_=otiles[6][:, H:])
            # group 7 (the true last): halves on ACT+DVE, stores on the two
            # rings whose queues are free at that point
            emit_epi(accs[7][:, :H], 7, True, otiles[7][:, :H])
            emit_epi(accs[7][:, H:], 7, False, otiles[7][:, H:])
            nc.scalar.dma_start(out=yT_d.ap()[7 * P :, MB : MB + H], in_=otiles[7][:, :H])
            nc.sync.dma_start(out=yT_d.ap()[7 * P :, MB + H :], in_=otiles[7][:, H:])

    if debarrier:
        # Drop the framework's entry all-engine barrier: every real
        # dependency already has a tile-emitted semaphore, and the barrier
        # serializes all engines behind the slowest pre-barrier stream.
        entry = nc.main_func.blocks[0]
        drop = [
            inst for inst in entry.instructions
            if type(inst).__name__ in ("InstDrain", "InstEventSemaphore")
        ]
        assert len(drop) == 11, [str(i)[:60] for i in drop]
        n_bar = sum("barrier_" in str(i) for i in drop)
        assert n_bar == 10, n_bar  # 5x(Drain+EvtSem w/ barrier sem) + bare PL Drain
        for inst in drop:
            entry.instructions.remove(inst)

        # Drop the SECOND exit all-engine barrier round (after the tile
        # RANGE_CLEAR): the walrus NEFF epilogue re-clears every semaphore
        # anyway, and concurrent zero-writes to already-zero sems are
        # benign. Keeps round 1 (which fences the RANGE_CLEAR against all
        # engines' last user ops) and the DMA-completion waits.
        end = nc.main_func.blocks[-1]
        isa_idx = max(
            i for i, inst in enumerate(end.instructions)
            if type(inst).__name__ == "InstISA"
        )
        tail = end.instructions[isa_idx + 1 :]
        assert len(tail) == 11, [str(i)[:60] for i in tail]
        assert all(
            type(i).__name__ in ("InstDrain", "InstEventSemaphore") for i in tail
        ), [str(i)[:60] for i in tail]
        n_bar2 = sum("barrier_" in str(i) for i in tail)
        assert n_bar2 == 10, n_bar2
        for inst in tail:
            end.instructions.remove(inst)

    nc.compile()
    return nc


def get_nc():
    if "nc" not in _CACHE:
        _CACHE["nc"] = build_bass()
    return _CACHE["nc"]


def make_in_maps(x, w, b):
    x = np.asarray(x, dtype=np.float32)
    w = np.asarray(w, dtype=np.float32)
    b = np.ascontiguousarray(b, dtype=np.float32)
    w_bf = np.ascontiguousarray(w.astype(ml_dtypes.bfloat16))
    xs = x.reshape(N_CORES, M, D_IN)
    return [
        {
            "xT": np.ascontiguousarray(xs[i].T.astype(ml_dtypes.bfloat16)),
            "w": w_bf,
            "b": b,
        }
        for i in range(N_CORES)
    ]


def gather_out(results):
    return np.concatenate(
        [results[i]["yT"].astype(np.float32).T for i in range(N_CORES)], axis=0
    )


def kernel(x, w, b):
    nc = get_nc()
    res = run_bass_kernel_spmd(nc, make_in_maps(x, w, b), core_ids=list(range(N_CORES)))
    return gather_out(res.results)


# revision 13
# speedup vs baseline: 1.1173x; 1.0211x over previous
"""Dense MLP forward (y = quantize(relu(x @ w + b))) on 8 TRN2 NeuronCores.

Strategy: pure data-parallel over the batch dim (1024 rows per core), w/b
replicated, no collectives. Host-side each core receives its x shard
*transposed* so the contraction dim lands on SBUF partitions with contiguous
DMA — zero on-chip transposes. Each core computes yT tiles:

  - matmuls in bf16 (x and w both rounded host-side; adds ~3e-3 rel err vs
    the 2e-2 gate). bf16 halves x's HBM traffic vs f32 and LDWEIGHTS gets
    FWL (4-xbus fast weight load), so the LDW fully hides under the 213ns
    N=512 moving stream; w chunks [128k,128n] stationary, xT chunks
    [128k,512m] moving, accumulating over k into all 8 PSUM banks; k-major
    wave order in band 0 so the PE starts as soon as the first chunks land;
    band 1 skewed so group stops stagger and evictions overlap matmuls.
  - startup: the framework's entry all-engine barrier is deleted from the
    IR; junk matmuls (N=256, bf16, first thing the PE runs) bridge the
    first-DMA window and release the PE HAM clock throttle (1.2->2.4GHz).
    Input DMAs split across the SP ring (x band-0 + first w pieces,
    interleaved) and ACT ring (bias + later w chunks) so descriptor
    generation (~0.6us per dma_start, serialized per sequencer) doesn't
    gate chunk arrival.
  - epilogue per [128n, 512m] tile: relu(psum + b) in one op (bias is
    per-partition in the transposed layout), groups alternating ACT/DVE so
    PSUM banks release in parallel; each store gen is emitted right after
    its epilogue. The last two groups run their epilogues in half-width
    pieces on BOTH engines concurrently and store the halves on separate
    DMA rings, shortening the post-last-matmul chain.

Host transposes each core's yT back and concatenates.
"""

import numpy as np
import ml_dtypes

import concourse.bacc as bacc
import concourse.tile as tile
from concourse import mybir
from concourse.bass_utils import run_bass_kernel_spmd

P = 128
B, D_IN, D_OUT = 8192, 1024, 1024
N_CORES = 8
M = B // N_CORES          # batch rows per core
KC = D_IN // P            # 8 k-chunks
NT = D_OUT // P           # 8 n-groups (PSUM partition tiles)
MB = 512                  # matmul moving free dim / PSUM bank width (fp32)
NUM_MB = M // MB          # 2 m-bands per core

N_WARMUP_MM = 13          # N=256 junk MMs filling the first-DMA window
DEBARRIER = True          # drop the entry-block all-engine barrier

F32 = mybir.dt.float32
BF16 = mybir.dt.bfloat16

_CACHE = {}


def build_bass(debarrier=DEBARRIER):
    nc = bacc.Bacc("TRN2", target_bir_lowering=False, debug=False)

    xT_d = nc.dram_tensor("xT", [D_IN, M], BF16, kind="ExternalInput")
    w_d = nc.dram_tensor("w", [D_IN, D_OUT], BF16, kind="ExternalInput")
    b_d = nc.dram_tensor("b", [D_OUT], F32, kind="ExternalInput")
    yT_d = nc.dram_tensor("yT", [D_OUT, M], BF16, kind="ExternalOutput")

    with tile.TileContext(nc) as tc:
        with (
            tc.tile_pool(name="const", bufs=1) as cst,
            tc.tile_pool(name="wx", bufs=1) as wx,
            tc.tile_pool(name="outp", bufs=16) as outp,
            tc.tile_pool(name="ps", bufs=1, space="PSUM") as ps,
        ):
            w_tiles = [wx.tile([P, D_OUT], BF16, tag=f"wc{c}", name=f"wc{c}") for c in range(KC)]
            x_tiles = [wx.tile([P, M], BF16, tag=f"xc{c}", name=f"xc{c}") for c in range(KC)]
            zt = cst.tile([P, 256], BF16, tag="warm_src")
            b_sb = cst.tile([P, NT], F32, tag="bias_raw")

            # ---- early ops ----
            # zt is junk-matmul fodder; it must be written once for the tile
            # allocator, so memset it on DVE (idle until the epilogues).
            nc.vector.memset(zt, 0.0)
            # A single DMA queue moves only ~135 GB/s while several are
            # active, so inputs spread across THREE queues (ACT + SP HWDGE,
            # gpsimd SWDGE) interleaved in need-order: each queue's local
            # order matches the k-major wave schedule so every wave's w
            # chunk and x band-0 piece land with slack. gpsimd leaves the
            # runtime preamble first, SP last (~1.3us later).
            def wpc(c, sl=slice(None)):
                return (w_tiles[c][:, sl], w_d.ap()[c * P : (c + 1) * P, sl])

            def xpc(c, sl):
                return (x_tiles[c][:, sl], xT_d.ap()[c * P : (c + 1) * P, sl])

            lo, hi = slice(None, MB), slice(MB, None)
            gp_pieces = [xpc(0, lo), wpc(1), xpc(2, lo), wpc(4), xpc(5, lo), wpc(7), xpc(7, lo)]
            act_pieces = [wpc(0, lo), wpc(0, hi), wpc(2), xpc(3, lo), wpc(6)]
            sp_pieces = [xpc(1, lo), wpc(3), xpc(4, lo), wpc(5), xpc(6, lo)] + [
                xpc(c, hi) for c in range(KC)
            ]
            for out, in_ in gp_pieces:
                nc.gpsimd.dma_start(out=out, in_=in_)
            for out, in_ in act_pieces:
                nc.scalar.dma_start(out=out, in_=in_)
            nc.scalar.dma_start(out=b_sb, in_=b_d.ap().rearrange("(c p) -> p c", p=P))

            # PE warm-up on junk data (uninitialized SBUF — the junk PSUM
            # output is overwritten with start=True, so values don't matter)
            # while the first input DMAs stream in.
            warm_ps = ps.tile([P, MB], F32, tag="acc7")
            for _ in range(N_WARMUP_MM):
                nc.tensor.matmul(warm_ps[:, :256], zt[:, :P], zt, start=True, stop=True)

            for out, in_ in sp_pieces:
                nc.sync.dma_start(out=out, in_=in_)

            def emit_mm(accs, mb, nt, c):
                nc.tensor.matmul(
                    accs[nt],
                    w_tiles[c][:, nt * P : (nt + 1) * P],
                    x_tiles[c][:, mb * MB : (mb + 1) * MB],
                    start=(c == 0),
                    stop=(c == KC - 1),
                )

            def emit_epi(acc_sl, nt, on_act, o_sl):
                # relu(psum + b) -> bf16; bias varies along partitions here
                if on_act:
                    nc.scalar.activation(
                        o_sl, acc_sl, mybir.ActivationFunctionType.Relu,
                        bias=b_sb[:, nt : nt + 1], scale=1.0,
                    )
                else:
                    nc.vector.tensor_scalar(
                        o_sl, acc_sl, b_sb[:, nt : nt + 1], 0.0,
                        mybir.AluOpType.add, mybir.AluOpType.max,
                    )

            # ---- band 0: k-major waves, 8 MMs per arriving chunk ----
            accs = [ps.tile([P, MB], F32, tag=f"acc{nt}", name=f"acc{nt}") for nt in range(NT)]
            for c in range(KC):
                for nt in range(NT):
                    emit_mm(accs, 0, nt, c)
            otiles = []
            for nt in range(NT):
                o = outp.tile([P, MB], BF16, tag="otile")
                otiles.append(o)
                emit_epi(accs[nt], nt, nt % 2 == 0, o)
            for nt in range(NT):
                ring = nc.scalar if nt % 2 == 0 else nc.sync
                ring.dma_start(
                    out=yT_d.ap()[nt * P : (nt + 1) * P, :MB], in_=otiles[nt]
                )

            # ---- band 1: skewed waves (group nt runs chunk c at wave
            # t=nt+c) so stops stagger and evictions overlap matmuls; the
            # last two groups split their epilogues across ACT+DVE and their
            # stores across rings so the post-last-matmul chain is short. ----
            accs = [ps.tile([P, MB], F32, tag=f"acc{nt}", name=f"acc{nt}") for nt in range(NT)]
            H = MB // 2
            for t in range(KC + NT - 1):
                for nt in range(NT):
                    c = t - nt
                    if 0 <= c < KC:
                        emit_mm(accs, 1, nt, c)
            otiles = [outp.tile([P, MB], BF16, tag="otile", name=f"ot1_{i}") for i in range(NT)]
            for nt in range(NT - 2):
                emit_epi(accs[nt], nt, nt % 2 == 0, otiles[nt])
                ring = nc.sync if nt % 2 == 0 else nc.scalar
                ring.dma_start(
                    out=yT_d.ap()[nt * P : (nt + 1) * P, MB:], in_=otiles[nt]
                )
            # group 6: halves on DVE+ACT, stores on sync+scalar rings
            emit_epi(accs[6][:, :H], 6, False, otiles[6][:, :H])
            emit_epi(accs[6][:, H:], 6, True, otiles[6][:, H:])
            nc.sync.dma_start(out=yT_d.ap()[6 * P : 7 * P, MB : MB + H], in_=otiles[6][:, :H])
            nc.scalar.dma_start(out=yT_d.ap()[6 * P : 7 * P, MB + H :], in_=otiles[6][:, H:])
            # group 7 (the true last): quarter-width epilogues alternating
            # ACT/DVE so both halves of the final store unblock ~250ns after
            # the last matmul's PSUM is readable; half-stores on both rings.
            Q = MB // 4
            for qi in range(4):
                sl = slice(qi * Q, (qi + 1) * Q)
                emit_epi(accs[7][:, sl], 7, qi % 2 == 0, otiles[7][:, sl])
            nc.sync.dma_start(out=yT_d.ap()[7 * P :, MB : MB + H], in_=otiles[7][:, :H])
            nc.scalar.dma_start(out=yT_d.ap()[7 * P :, MB + H :], in_=otiles[7][:, H:])

    if debarrier:
        # Drop the framework's entry all-engine barrier: every real
        # dependency already has a tile-emitted semaphore, and the barrier
        # serializes all engines behind the slowest pre-barrier stream.
        entry = nc.main_func.blocks[0]
        drop = [
            inst for inst in entry.instructions
            if type(inst).__name__ in ("InstDrain", "InstEventSemaphore")
        ]
        assert len(drop) == 11, [str(i)[:60] for i in drop]
        n_bar = sum("barrier_" in str(i) for i in drop)
        assert n_bar == 10, n_bar  # 5x(Drain+EvtSem w/ barrier sem) + bare PL Drain
        for inst in drop:
            entry.instructions.remove(inst)

        # Drop the SECOND exit all-engine barrier round (after the tile
        # RANGE_CLEAR): the walrus NEFF epilogue re-clears every semaphore
        # anyway, and concurrent zero-writes to already-zero sems are
        # benign. Keeps round 1 (which fences the RANGE_CLEAR against all
        # engines' last user ops) and the DMA-completion waits.
        end = nc.main_func.blocks[-1]
        isa_idx = max(
            i for i, inst in enumerate(end.instructions)
            if type(inst).__name__ == "InstISA"
        )
        tail = end.instructions[isa_idx + 1 :]
        assert len(tail) == 11, [str(i)[:60] for i in tail]
        assert all(
            type(i).__name__ in ("InstDrain", "InstEventSemaphore") for i in tail
        ), [str(i)[:60] for i in tail]
        n_bar2 = sum("barrier_" in str(i) for i in tail)
        assert n_bar2 == 10, n_bar2
        for inst in tail:
            end.instructions.remove(inst)

    nc.compile()
    return nc


def get_nc():
    if "nc" not in _CACHE:
        _CACHE["nc"] = build_bass()
    return _CACHE["nc"]


def make_in_maps(x, w, b):
    x = np.asarray(x, dtype=np.float32)
    w = np.asarray(w, dtype=np.float32)
    b = np.ascontiguousarray(b, dtype=np.float32)
    w_bf = np.ascontiguousarray(w.astype(ml_dtypes.bfloat16))
    xs = x.reshape(N_CORES, M, D_IN)
    return [
        {
            "xT": np.ascontiguousarray(xs[i].T.astype(ml_dtypes.bfloat16)),
            "w": w_bf,
            "b": b,
        }
        for i in range(N_CORES)
    ]


def gather_out(results):
    return np.concatenate(
        [results[i]["yT"].astype(np.float32).T for i in range(N_CORES)], axis=0
    )


def kernel(x, w, b):
    nc = get_nc()
    res = run_bass_kernel_spmd(nc, make_in_maps(x, w, b), core_ids=list(range(N_CORES)))
    return gather_out(res.results)
